# revision 1
# baseline (speedup 1.0000x reference)
"""NomicBertAttention on 8 Trainium2 NeuronCores.

Sharding: 8-way head tensor-parallelism (2 heads/core, both batches),
AllToAll to re-shard ctx by sequence rows, then row-parallel out-proj +
residual + LayerNorm (each core owns 512 of the 4096 flattened rows).

Dataflow (per core, everything fp32r on the PE):
  phase A: q/k/v projections in transposed layout [feat, pos] with the
           bias folded in via a ones-row matmul; RoPE applied by fused
           DVE reads of the projection PSUM (cos-mul, 4 rotated sin-muls,
           add); v transposed to natural [t, d] via PE transpose and
           augmented with a ones column (softmax denominator).
  phase B: scoresT[t,s] = k'.q' (two heads packed in the 128-row PE array,
           K=64 each); exp on ACT straight out of PSUM into fp32r SBUF;
           ctx^T (+den row) accumulated over t in PSUM; normalize via
           DVE reciprocal + GPSIMD partition_broadcast + DVE multiply.
  phase C: AllToAll of ctx^T blocks; out-proj (lhsT = gathered ctx^T);
           residual add + LayerNorm fused into DVE/ACT passes.
"""

import numpy as np
import concourse.bacc as bacc
import concourse.mybir as mybir
import concourse.tile as tile
from concourse.bass_utils import run_bass_kernel_spmd
from concourse.masks import make_identity

F32 = mybir.dt.float32
F32R = mybir.dt.float32r
MULT = mybir.AluOpType.mult
ADD = mybir.AluOpType.add
SUB = mybir.AluOpType.subtract
BYPASS = mybir.AluOpType.bypass
EXP = mybir.ActivationFunctionType.Exp
SQRT = mybir.ActivationFunctionType.Sqrt
IDENT = mybir.ActivationFunctionType.Identity

B, S, D, H, HD = 2, 2048, 1024, 16, 64
NC = 8
HPC = H // NC          # 2 heads per core
F = HPC * HD           # 128 projected features per core
ROWS = B * S // NC     # 512 output rows per core
NSEQ = B * S           # 4096 flattened rows
EPS = 1e-12

LAST_RESULTS = None


def _build():
    nc = bacc.Bacc("TRN2", target_bir_lowering=False, debug=False, num_devices=NC)

    xaT = nc.dram_tensor("xaT", [D + 1, NSEQ], F32R, kind="ExternalInput")
    wq = nc.dram_tensor("wq", [D, F], F32R, kind="ExternalInput")
    wk = nc.dram_tensor("wk", [D, F], F32R, kind="ExternalInput")
    wv = nc.dram_tensor("wv", [D, F], F32R, kind="ExternalInput")
    wqb = nc.dram_tensor("wqb", [1, F], F32R, kind="ExternalInput")
    wkb = nc.dram_tensor("wkb", [1, F], F32R, kind="ExternalInput")
    wvb = nc.dram_tensor("wvb", [1, F], F32R, kind="ExternalInput")
    woT = nc.dram_tensor("woT", [D, D], F32R, kind="ExternalInput")
    cs2d = nc.dram_tensor("cs2", [128, S], F32, kind="ExternalInput")
    sn2d = nc.dram_tensor("sn2", [128, S], F32, kind="ExternalInput")
    residd = nc.dram_tensor("resid", [ROWS, D], F32, kind="ExternalInput")
    lnwd = nc.dram_tensor("lnw", [128, D], F32, kind="ExternalInput")
    lnbd = nc.dram_tensor("lnb", [128, D], F32, kind="ExternalInput")
    outd = nc.dram_tensor("out", [ROWS, D], F32, kind="ExternalOutput")
    import os as _os0
    _dump = bool(_os0.environ.get("KD_DUMP"))
    if _dump:
        qSo = nc.dram_tensor("qSo", [128, NSEQ], F32, kind="ExternalOutput")
        kSo = nc.dram_tensor("kSo", [128, NSEQ], F32, kind="ExternalOutput")
        cfo = nc.dram_tensor("cfo", [128, NSEQ], F32, kind="ExternalOutput")
        vao = nc.dram_tensor("vao", [128, 65 * NSEQ // 128], F32, kind="ExternalOutput")

    NT = NSEQ // 128    # 32 global t-chunks
    TB = S // 128       # 16 t-chunks per batch

    with tile.TileContext(nc) as tc:
        with (
            tc.tile_pool(name="qk", bufs=1) as qkpool,
            tc.tile_pool(name="exps", bufs=3) as epool,
            tc.tile_pool(name="ctxp", bufs=2) as ctxpool,
            tc.tile_pool(name="bcastp", bufs=2) as bpool,
            tc.tile_pool(name="small", bufs=4) as spool,
            tc.tile_pool(name="psS", bufs=2, space="PSUM") as psS,
            tc.tile_pool(name="psC", bufs=4, space="PSUM") as psC,
            tc.tile_pool(name="dram", bufs=1, space="DRAM") as dpool,
        ):
            # resident RoPE'd projections [feat(2 heads), pos]
            qS = qkpool.tile([128, NSEQ], F32R, tag="qS")
            kS = qkpool.tile([128, NSEQ], F32R, tag="kS")
            # v natural, ones-augmented: per head [128(t), 65*NT]
            vaug = [
                qkpool.tile([128, 65 * NT], F32R, tag=f"vaug{h}", name=f"vaug{h}")
                for h in range(HPC)
            ]
            for h in range(HPC):
                ones_view = vaug[h][:].rearrange("p (t c) -> p t c", c=65)[:, :, 64:65]
                nc.vector.memset(ones_view.bitcast(F32), 1.0)

            # ---- phase A: projections + RoPE + v transpose (scoped pools)
            with (
                tc.tile_pool(name="wpool", bufs=1) as wpool,
                tc.tile_pool(name="xpool", bufs=2) as xpool,
                tc.tile_pool(name="rope", bufs=2) as rpool,
                tc.tile_pool(name="vv", bufs=2) as vpool,
            ):
              cs2 = wpool.tile([128, S], F32, tag="cs2")
              sn2 = wpool.tile([128, S], F32, tag="sn2")
              nc.sync.dma_start(cs2[:], cs2d[:])
              nc.sync.dma_start(sn2[:], sn2d[:])
              ident = wpool.tile([128, 128], F32, tag="ident")
              make_identity(nc, ident[:])
              wsb = {}
              wbias = {}
              for name, dram_w, dram_b in (
                  ("q", wq, wqb),
                  ("k", wk, wkb),
                  ("v", wv, wvb),
              ):
                  wt = wpool.tile([128, D], F32R, tag=f"w{name}", name=f"w{name}")
                  for k in range(8):
                      nc.sync.dma_start(
                          wt[:, 128 * k : 128 * (k + 1)],
                          dram_w[128 * k : 128 * (k + 1), :],
                      )
                  wsb[name] = wt
                  bt = wpool.tile([1, F], F32R, tag=f"wb{name}", name=f"wb{name}")
                  nc.sync.dma_start(bt[:], dram_b[:])
                  wbias[name] = bt
              for g in range(NSEQ // 512):
                gs, ge = 512 * g, 512 * (g + 1)
                cg = (512 * g) % S
                xg = xpool.tile([128, 4096], F32R, tag="xg")
                xone = xpool.tile([1, 512], F32R, tag="xone")
                for k in range(8):
                    nc.sync.dma_start(
                        xg[:, 512 * k : 512 * (k + 1)],
                        xaT[128 * k : 128 * (k + 1), gs:ge],
                    )
                nc.sync.dma_start(xone[:], xaT[D : D + 1, gs:ge])

                for name in ("q", "k", "v"):
                    pp = psS.tile([128, 512], F32, tag="sc")
                    for k in range(8):
                        nc.tensor.matmul(
                            pp[:],
                            wsb[name][:, 128 * k : 128 * (k + 1)],
                            xg[:, 512 * k : 512 * (k + 1)],
                            start=(k == 0),
                            stop=False,
                        )
                    nc.tensor.matmul(pp[:], wbias[name][:], xone[:], start=False, stop=True)

                    if name in ("q", "k"):
                        dst = qS if name == "q" else kS
                        tcos = rpool.tile([128, 512], F32, tag="tcos")
                        nc.vector.tensor_tensor(
                            out=tcos[:], in0=pp[:], in1=cs2[:, cg : cg + 512], op=MULT
                        )
                        tsin = rpool.tile([128, 512], F32, tag="tsin")
                        for h in range(2):
                            for x in range(2):
                                o0 = 64 * h + 32 * x
                                i0 = 64 * h + 32 * (1 - x)
                                nc.vector.tensor_tensor(
                                    out=tsin[o0 : o0 + 32, :],
                                    in0=pp[i0 : i0 + 32, :],
                                    in1=sn2[o0 : o0 + 32, cg : cg + 512],
                                    op=MULT,
                                )
                        nc.vector.tensor_tensor(
                            out=dst[:, gs:ge], in0=tcos[:], in1=tsin[:], op=ADD
                        )
                    else:
                        vTg = vpool.tile([128, 512], F32, tag="vTg")
                        nc.vector.tensor_copy(vTg[:], pp[:])
                        for sub in range(4):
                            trp = psC.tile([128, 128], F32, tag="cp")
                            nc.tensor.transpose(
                                trp[:],
                                vTg[:, 128 * sub : 128 * (sub + 1)],
                                ident[:],
                            )
                            tcg = 4 * g + sub
                            for h in range(HPC):
                                nc.vector.tensor_copy(
                                    vaug[h][:, 65 * tcg : 65 * tcg + 64],
                                    trp[:, 64 * h : 64 * (h + 1)],
                                )

            if _dump:
                nc.sync.dma_start(qSo[:], qS[:].bitcast(F32))
                nc.sync.dma_start(kSo[:], kS[:].bitcast(F32))
                nc.sync.dma_start(vao[:], vaug[0][:].bitcast(F32))

            # ---- A2A bounce buffers
            a2a_in = dpool.tile([NC, 128, 512], F32, tag="a2a_in")
            a2a_out = dpool.tile([NC, 128, 512], F32, tag="a2a_out")

            # ---- phase B: attention per (batch, 1024-wide s-window)
            import os as _os
            for b in range(0 if _os.environ.get("KD_SKIP_B") else B):
                for gw in range(2):
                    sw = S * b + 1024 * gw
                    j0 = 4 * b + 2 * gw
                    cps = {}
                    for h in range(HPC):
                        for half in range(2):
                            cps[h, half] = psC.tile([65, 512], F32, tag="cp", name=f"cp_{h}_{half}")
                    for tcl in range(TB):
                        tg = S * b + 128 * tcl
                        tcg = TB * b + tcl
                        for h in range(HPC):
                            hs_, he = 64 * h, 64 * (h + 1)
                            sc = psS.tile([128, 1024], F32, tag="sc")
                            for half in range(2):
                                s0 = sw + 512 * half
                                nc.tensor.matmul(
                                    sc[:, 512 * half : 512 * (half + 1)],
                                    kS[hs_:he, tg : tg + 128],
                                    qS[hs_:he, s0 : s0 + 512],
                                    start=True,
                                    stop=True,
                                )
                            ex = epool.tile([128, 1024], F32R, tag="ex")
                            nc.scalar.activation(ex[:], sc[:], EXP)
                            for half in range(2):
                                nc.tensor.matmul(
                                    cps[h, half][:],
                                    vaug[h][:, 65 * tcg : 65 * tcg + 65],
                                    ex[:, 512 * half : 512 * (half + 1)],
                                    start=(tcl == 0),
                                    stop=(tcl == TB - 1),
                                )
                    for half in range(2):
                        j = j0 + half
                        ctile = ctxpool.tile([128, 512], F32R, tag="ctile")
                        for h in range(HPC):
                            rden = spool.tile([1, 512], F32, tag="rden")
                            nc.vector.reciprocal(rden[:], cps[h, half][64:65, :])
                            bc = bpool.tile([64, 512], F32, tag="bc")
                            nc.gpsimd.partition_broadcast(bc[:], rden[:])
                            nc.vector.tensor_tensor(
                                out=ctile[64 * h : 64 * (h + 1), :],
                                in0=cps[h, half][0:64, :],
                                in1=bc[:],
                                op=MULT,
                            )
                        nc.sync.dma_start(a2a_in[j], ctile[:].bitcast(F32))

            # ---- phase C: A2A + out-proj + residual + LayerNorm
            skip_c = bool(_os.environ.get("KD_SKIP_C"))
            if _os.environ.get("KDEBUG_NOA2A") and not skip_c:
                nc.sync.dma_start(a2a_out[:], a2a_in[:])
            elif not skip_c:
                nc.gpsimd.collective_compute(
                    "AllToAll",
                    BYPASS,
                    replica_groups=[list(range(NC))],
                    ins=[a2a_in[:]],
                    outs=[a2a_out[:]],
                )
            if not skip_c:
              with (
                tc.tile_pool(name="wop", bufs=16) as wopool,
                tc.tile_pool(name="tail", bufs=2) as lpool,
              ):
                ctxF = lpool.tile([128, 4096], F32R, tag="ctxF", bufs=1)
                if _os.environ.get("KD_CTXF_ZERO"):
                    nc.vector.memset(ctxF[:].bitcast(F32), 0.125)
                else:
                    for i in range(NC):
                        nc.sync.dma_start(
                            ctxF[:, 512 * i : 512 * (i + 1)].bitcast(F32), a2a_out[i]
                        )

                if _dump:
                    nc.sync.dma_start(cfo[:], ctxF[:].bitcast(F32))
                lnw = lpool.tile([128, D], F32, tag="lnw", bufs=1)
                lnb = lpool.tile([128, D], F32, tag="lnb", bufs=1)
                nc.sync.dma_start(lnw[:], lnwd[:])
                nc.sync.dma_start(lnb[:], lnbd[:])

                won = {}
                for n in range(2):
                    for k in range(8):
                        wot = wopool.tile([128, 512], F32R, tag="wo")
                        nc.sync.dma_start(
                            wot[:], woT[128 * k : 128 * (k + 1), 512 * n : 512 * (n + 1)]
                        )
                        won[n, k] = wot

                eps_t = spool.tile([128, 1], F32, tag="eps_t", bufs=1)
                nc.vector.memset(eps_t[:], EPS)
                for m in range(0) if _os.environ.get("KD_ONLY_LOADS") else range(4):
                    rsb = lpool.tile([128, D], F32, tag="rsb")
                    nc.sync.dma_start(rsb[:], residd[128 * m : 128 * (m + 1), :])
                    osb = lpool.tile([128, D], F32, tag="osb")
                    acc_s = spool.tile([128, 1], F32, tag="acc_s")
                    acc_q = spool.tile([128, 1], F32, tag="acc_q")
                    for n in range(2):
                        op = psS.tile([128, 512], F32, tag="sc")
                        for k in range(8):
                            nc.tensor.matmul(
                                op[:],
                                ctxF[:, 512 * k + 128 * m : 512 * k + 128 * (m + 1)],
                                won[n, k][:],
                                start=(k == 0),
                                stop=(k == 7),
                            )
                        nc.vector.tensor_tensor(
                            out=osb[:, 512 * n : 512 * (n + 1)],
                            in0=op[:],
                            in1=rsb[:, 512 * n : 512 * (n + 1)],
                            op=ADD,
                        )
                    nc.vector.tensor_reduce(
                        acc_s[:], osb[:], mybir.AxisListType.X, ADD
                    )
                    scr = lpool.tile([128, D], F32, tag="scr")
                    nc.vector.tensor_tensor(out=scr[:], in0=osb[:], in1=osb[:], op=MULT)
                    nc.vector.tensor_reduce(
                        acc_q[:], scr[:], mybir.AxisListType.X, ADD
                    )
                    mean = spool.tile([128, 1], F32, tag="mean")
                    nc.vector.tensor_scalar(mean[:], acc_s[:], 1.0 / D, None, MULT)
                    msq = spool.tile([128, 1], F32, tag="msq")
                    nc.vector.tensor_scalar(msq[:], acc_q[:], 1.0 / D, None, MULT)
                    m2 = spool.tile([128, 1], F32, tag="m2")
                    nc.vector.tensor_tensor(out=m2[:], in0=mean[:], in1=mean[:], op=MULT)
                    var = spool.tile([128, 1], F32, tag="var")
                    nc.vector.tensor_tensor(out=var[:], in0=msq[:], in1=m2[:], op=SUB)
                    sdt = spool.tile([128, 1], F32, tag="sdt")
                    nc.scalar.activation(sdt[:], var[:], SQRT, bias=eps_t[:])
                    rstd = spool.tile([128, 1], F32, tag="rstd")
                    nc.vector.reciprocal(rstd[:], sdt[:])
                    mr = spool.tile([128, 1], F32, tag="mr")
                    nc.vector.tensor_tensor(out=mr[:], in0=mean[:], in1=rstd[:], op=MULT)
                    negmr = spool.tile([128, 1], F32, tag="negmr")
                    nc.vector.tensor_scalar(negmr[:], mr[:], -1.0, None, MULT)
                    onrm = lpool.tile([128, D], F32, tag="onrm")
                    nc.scalar.activation(
                        onrm[:], osb[:], IDENT, bias=negmr[:], scale=rstd[:]
                    )
                    ow = lpool.tile([128, D], F32, tag="ow")
                    nc.vector.tensor_tensor(out=ow[:], in0=onrm[:], in1=lnw[:], op=MULT)
                    ofin = lpool.tile([128, D], F32, tag="ofin")
                    nc.vector.tensor_tensor(out=ofin[:], in0=ow[:], in1=lnb[:], op=ADD)
                    nc.sync.dma_start(outd[128 * m : 128 * (m + 1), :], ofin[:])

    nc.finalize()
    return nc


def kernel(hidden_states, cos, sin, Wq, bq, Wk, bk, Wv, bv, Wo, bo, ln_w, ln_b):
    global LAST_RESULTS
    hs = np.ascontiguousarray(np.asarray(hidden_states, np.float32).reshape(NSEQ, D))
    cos = np.asarray(cos, np.float32)
    sin = np.asarray(sin, np.float32)
    Wq = np.asarray(Wq, np.float32)
    bq = np.asarray(bq, np.float32)
    Wk = np.asarray(Wk, np.float32)
    bk = np.asarray(bk, np.float32)
    Wv = np.asarray(Wv, np.float32)
    bv = np.asarray(bv, np.float32)
    Wo = np.asarray(Wo, np.float32)
    bo = np.asarray(bo, np.float32)
    ln_w = np.asarray(ln_w, np.float32)
    ln_b = np.asarray(ln_b, np.float32)

    xaT = np.ascontiguousarray(
        np.concatenate([hs.T, np.ones((1, NSEQ), np.float32)], axis=0)
    )
    cosT = np.ascontiguousarray(cos.T)                      # [64, S]
    snT = sin.T
    snsgn = np.concatenate([-snT[:32], snT[32:]], axis=0)   # [64, S]
    cs2 = np.ascontiguousarray(np.concatenate([cosT, cosT], axis=0))
    sn2 = np.ascontiguousarray(np.concatenate([snsgn, snsgn], axis=0))
    lnw_t = np.ascontiguousarray(np.tile(ln_w[None, :], (128, 1)))
    lnb_t = np.ascontiguousarray(np.tile(ln_b[None, :], (128, 1)))
    woT = np.ascontiguousarray(Wo.T)                        # [din, dout]
    wq_s = (Wq / 64.0).T                                    # fold SCALING^2
    bq_s = bq / 64.0

    in_maps = []
    for c in range(NC):
        fs = slice(F * c, F * (c + 1))
        rs = slice(ROWS * c, ROWS * (c + 1))
        in_maps.append(
            {
                "xaT": xaT,
                "wq": np.ascontiguousarray(wq_s[:, fs]),
                "wk": np.ascontiguousarray(Wk.T[:, fs]),
                "wv": np.ascontiguousarray(Wv.T[:, fs]),
                "wqb": np.ascontiguousarray(bq_s[None, fs]),
                "wkb": np.ascontiguousarray(bk[None, fs]),
                "wvb": np.ascontiguousarray(bv[None, fs]),
                "woT": woT,
                "cs2": cs2,
                "sn2": sn2,
                "resid": np.ascontiguousarray(hs[rs] + bo[None, :]),
                "lnw": lnw_t,
                "lnb": lnb_t,
            }
        )

    nc = _build()
    LAST_RESULTS = run_bass_kernel_spmd(nc, in_maps, core_ids=list(range(NC)))
    out = np.concatenate([LAST_RESULTS.results[c]["out"] for c in range(NC)], axis=0)
    return out.reshape(B, S, D)



# revision 16
# speedup vs baseline: 1.3667x; 1.3667x over previous
"""NomicBertAttention on 8 Trainium2 NeuronCores.

Sharding: 8-way head tensor-parallelism (2 heads/core, both batches).
Per 1024-column window of the flattened (b,s) axis, an fp16 AllToAll
re-shards ctx^T by sequence rows (each core owns one 128-row block per
window), and the row-parallel out-proj + residual + LayerNorm for that
window is software-pipelined behind the next window's attention.

All PE matmuls run in fp16 (fp32 PSUM accumulation): projections with
the bias folded into the ACT-engine PSUM evacuation, RoPE rotate-half
via a constant +-1 rotation matrix on the PE, scores/ctx per head with
a ones-column in V giving the softmax denominator for free, and the
out-projection from the gathered fp16 ctx^T. Softmax denominators use
reciprocal_approx_fast; LayerNorm rstd uses a magic-constant rsqrt with
two Newton steps on the DVE so the ACT engine stays on the Exp table.
"""

import numpy as np
import concourse.bacc as bacc
import concourse.mybir as mybir
import concourse.tile as tile
from concourse.bass_utils import run_bass_kernel_spmd
from concourse.masks import make_identity

F32 = mybir.dt.float32
F16 = mybir.dt.float16
I32 = mybir.dt.int32
MULT = mybir.AluOpType.mult
ADD = mybir.AluOpType.add
SUB = mybir.AluOpType.subtract
XOR = mybir.AluOpType.bitwise_xor
SHR = mybir.AluOpType.arith_shift_right
BYPASS = mybir.AluOpType.bypass
EXP = mybir.ActivationFunctionType.Exp
IDENT = mybir.ActivationFunctionType.Identity

B, S, D, H, HD = 2, 2048, 1024, 16, 64
NC = 8
HPC = H // NC          # 2 heads per core
F = HPC * HD           # 128 projected features per core
NSEQ = B * S           # 4096 flattened rows
ROWS = NSEQ // NC      # 512 output rows per core (4 blocks of 128)
NW = 4                 # 1024-column windows
TB = S // 128          # 16 t-chunks per batch
EPS = 1e-12
RSQRT_MAGIC = 0x5F3759DF

LAST_RESULTS = None


def _build():
    nc = bacc.Bacc("TRN2", target_bir_lowering=False, debug=False, num_devices=NC)

    xT = nc.dram_tensor("xT", [D, NSEQ], F16, kind="ExternalInput")
    wq = nc.dram_tensor("wq", [D, F], F16, kind="ExternalInput")
    wk = nc.dram_tensor("wk", [D, F], F16, kind="ExternalInput")
    wv = nc.dram_tensor("wv", [D, F], F16, kind="ExternalInput")
    bqd = nc.dram_tensor("bq", [F, 1], F32, kind="ExternalInput")
    bkd = nc.dram_tensor("bk", [F, 1], F32, kind="ExternalInput")
    bvd = nc.dram_tensor("bv", [F, 1], F32, kind="ExternalInput")
    rtd = nc.dram_tensor("rt", [128, 128], F16, kind="ExternalInput")
    woT = nc.dram_tensor("woT", [D, D], F16, kind="ExternalInput")
    cs2d = nc.dram_tensor("cs2", [128, S], F16, kind="ExternalInput")
    sn2d = nc.dram_tensor("sn2", [128, S], F16, kind="ExternalInput")
    residd = nc.dram_tensor("resid", [ROWS, D], F32, kind="ExternalInput")
    lnwd = nc.dram_tensor("lnw", [128, D], F32, kind="ExternalInput")
    lnbd = nc.dram_tensor("lnb", [128, D], F32, kind="ExternalInput")
    outd = nc.dram_tensor("out", [ROWS, D], F32, kind="ExternalOutput")
    import os as _os0
    _dump = bool(_os0.environ.get("KD_DUMP"))
    if _dump:
        qSo = nc.dram_tensor("qSo", [128, NSEQ], F16, kind="ExternalOutput")
        kSo = nc.dram_tensor("kSo", [128, NSEQ], F16, kind="ExternalOutput")
        vao = nc.dram_tensor("vao", [128, 2 * TB * HPC * (HD + 1)], F16, kind="ExternalOutput")
        cto = nc.dram_tensor("cto", [NW, 128, 1024], F16, kind="ExternalOutput")
        cfo = nc.dram_tensor("cfo", [NW, 128, 1024], F16, kind="ExternalOutput")
        oso = nc.dram_tensor("oso", [NW, 128, D], F32, kind="ExternalOutput")

    with tile.TileContext(nc) as tc:
        with (
            tc.tile_pool(name="qk", bufs=1) as qkpool,
            tc.tile_pool(name="wpool", bufs=1) as wpool,
            tc.tile_pool(name="xpool", bufs=2) as xpool,
            tc.tile_pool(name="rope", bufs=2) as rpool,
            tc.tile_pool(name="exps", bufs=3) as epool,
            tc.tile_pool(name="ctxp", bufs=2) as ctxpool,
            tc.tile_pool(name="bcastp", bufs=2) as bpool,
            tc.tile_pool(name="small", bufs=4) as spool,
            tc.tile_pool(name="tail", bufs=2) as lpool,
            tc.tile_pool(name="psS", bufs=2, space="PSUM") as psS,
            tc.tile_pool(name="psC", bufs=4, space="PSUM") as psC,
            tc.tile_pool(name="dram", bufs=1, space="DRAM") as dpool,
        ):
            # ---- resident tensors
            qS = qkpool.tile([128, NSEQ], F16, tag="qS")
            kS = qkpool.tile([128, NSEQ], F16, tag="kS")
            # v natural + ones column: [t-part, tcg, head, 64+1]
            vaug = qkpool.tile([128, 2 * TB, HPC, HD + 1], F16, tag="vaug")
            nc.vector.memset(vaug[:, :, :, HD : HD + 1], 1.0)

            cs2 = wpool.tile([128, S], F16, tag="cs2")
            sn2 = wpool.tile([128, S], F16, tag="sn2")
            nc.sync.dma_start(cs2[:], cs2d[:])
            nc.sync.dma_start(sn2[:], sn2d[:])
            rt = wpool.tile([128, 128], F16, tag="rt")
            nc.sync.dma_start(rt[:], rtd[:])
            ident = wpool.tile([128, 128], F16, tag="ident")
            make_identity(nc, ident[:])
            wsb = {}
            bias_t = {}
            for name, dram_w, dram_b in (("q", wq, bqd), ("k", wk, bkd), ("v", wv, bvd)):
                wt = wpool.tile([128, D], F16, tag=f"w{name}", name=f"w{name}")
                for k in range(8):
                    nc.sync.dma_start(
                        wt[:, 128 * k : 128 * (k + 1)],
                        dram_w[128 * k : 128 * (k + 1), :],
                    )
                wsb[name] = wt
                bt = wpool.tile([128, 1], F32, tag=f"b{name}", name=f"b{name}")
                nc.sync.dma_start(bt[:], dram_b[:])
                bias_t[name] = bt
            won = {}
            for n in range(2):
                for k in range(8):
                    wot = wpool.tile([128, 512], F16, tag=f"wo{n}{k}", name=f"wo{n}{k}")
                    nc.sync.dma_start(
                        wot[:], woT[128 * k : 128 * (k + 1), 512 * n : 512 * (n + 1)]
                    )
                    won[n, k] = wot
            lnw = wpool.tile([128, D], F32, tag="lnw")
            lnb = wpool.tile([128, D], F32, tag="lnb")
            nc.sync.dma_start(lnw[:], lnwd[:])
            nc.sync.dma_start(lnb[:], lnbd[:])

            a2a_in = {}
            a2a_out = {}
            for w in range(NW):
                a2a_in[w] = dpool.tile([NC, 128, 128], F16, tag=f"a2a_in{w}", name=f"a2a_in{w}")
                a2a_out[w] = dpool.tile([NC, 128, 128], F16, tag=f"a2a_out{w}", name=f"a2a_out{w}")

            # ---- phase A: projections + RoPE + v transpose for groups [g0, g1)
            def phase_a(g0, g1):
                for g in range(g0, g1):
                    gs = 512 * g
                    cg = gs % S
                    xg = xpool.tile([128, 4096], F16, tag="xg")
                    for k in range(8):
                        nc.sync.dma_start(
                            xg[:, 512 * k : 512 * (k + 1)],
                            xT[128 * k : 128 * (k + 1), gs : gs + 512],
                        )
                    for name in ("q", "k", "v"):
                        pp = psC.tile([128, 512], F32, tag="cp", name="pp")
                        for k in range(8):
                            nc.tensor.matmul(
                                pp[:],
                                wsb[name][:, 128 * k : 128 * (k + 1)],
                                xg[:, 512 * k : 512 * (k + 1)],
                                start=(k == 0),
                                stop=(k == 7),
                            )
                        plain = rpool.tile([128, 512], F16, tag="plain")
                        nc.scalar.activation(plain[:], pp[:], IDENT, bias=bias_t[name][:])
                        if name in ("q", "k"):
                            rotp = psC.tile([128, 512], F32, tag="cp", name="rotp")
                            nc.tensor.matmul(rotp[:], rt[:], plain[:], start=True, stop=True)
                            tc_ = rpool.tile([128, 512], F16, tag="tc")
                            nc.vector.tensor_tensor(
                                out=tc_[:], in0=plain[:], in1=cs2[:, cg : cg + 512], op=MULT
                            )
                            ts_ = rpool.tile([128, 512], F16, tag="ts")
                            nc.vector.tensor_tensor(
                                out=ts_[:], in0=rotp[:], in1=sn2[:, cg : cg + 512], op=MULT
                            )
                            dst = qS if name == "q" else kS
                            nc.gpsimd.tensor_tensor(
                                out=dst[:, gs : gs + 512], in0=tc_[:], in1=ts_[:], op=ADD
                            )
                        else:
                            for sub in range(4):
                                trp = psC.tile([128, 128], F16, tag="cp", name="trp")
                                nc.tensor.transpose(
                                    trp[:], plain[:, 128 * sub : 128 * (sub + 1)], ident[:]
                                )
                                tcg = 4 * g + sub
                                nc.vector.tensor_copy(
                                    vaug[:, tcg, :, 0:HD],
                                    trp[:].rearrange("p (h d) -> p h d", h=HPC),
                                )

            # ---- phase B: attention for window w (1024 s-columns), then
            # normalize + AllToAll launch (out-proj deferred to phase_c)
            def phase_b(w):
                b = w // 2
                sw = 1024 * w
                cps = {}
                for h in range(HPC):
                    for half in range(2):
                        cps[h, half] = psC.tile(
                            [HD + 1, 512], F32, tag="cp", name=f"cps_{h}_{half}"
                        )
                for tcl in range(TB):
                    tg = S * b + 128 * tcl
                    tcg = TB * b + tcl
                    for h in range(HPC):
                        hs_, he = HD * h, HD * (h + 1)
                        sc = psS.tile([128, 1024], F32, tag="sc")
                        for half in range(2):
                            s0 = sw + 512 * half
                            nc.tensor.matmul(
                                sc[:, 512 * half : 512 * (half + 1)],
                                kS[hs_:he, tg : tg + 128],
                                qS[hs_:he, s0 : s0 + 512],
                                start=True,
                                stop=True,
                            )
                        ex = epool.tile([128, 1024], F16, tag="ex")
                        nc.scalar.activation(ex[:], sc[:], EXP)
                        for half in range(2):
                            nc.tensor.matmul(
                                cps[h, half][:],
                                vaug[:, tcg, h, :],
                                ex[:, 512 * half : 512 * (half + 1)],
                                start=(tcl == 0),
                                stop=(tcl == TB - 1),
                            )
                # normalize ctx by the denominator row and ship via AllToAll
                ctile = ctxpool.tile([128, 1024], F16, tag="ctile")
                for h in range(HPC):
                    for half in range(2):
                        dsb = spool.tile([1, 512], F32, tag="dsb")
                        nc.vector.tensor_copy(dsb[:], cps[h, half][HD : HD + 1, :])
                        rden = spool.tile([1, 512], F32, tag="rden")
                        nc.vector.reciprocal_approx_fast(rden[:], dsb[:])
                        bc = bpool.tile([HD, 512], F32, tag="bc")
                        nc.gpsimd.partition_broadcast(bc[:], rden[:])
                        nc.vector.tensor_tensor(
                            out=ctile[HD * h : HD * (h + 1), 512 * half : 512 * (half + 1)],
                            in0=cps[h, half][0:HD, :],
                            in1=bc[:],
                            op=MULT,
                        )
                if _dump:
                    nc.sync.dma_start(cto[w], ctile[:])
                nc.sync.dma_start(
                    a2a_in[w][:].rearrange("c p s -> p c s"),
                    ctile[:].rearrange("p (c s) -> p c s", c=NC),
                )
                import os as _os
                if _os.environ.get("KD_NO_CC"):
                    nc.sync.dma_start(a2a_out[w][:], a2a_in[w][:])
                else:
                    nc.gpsimd.collective_compute(
                        "AllToAll",
                        BYPASS,
                        replica_groups=[list(range(NC))],
                        ins=[a2a_in[w][:]],
                        outs=[a2a_out[w][:]],
                    )
                rsb = lpool.tile([128, D], F32, tag="rsb", name=f"rsb{w}")
                nc.sync.dma_start(rsb[:], residd[128 * w : 128 * (w + 1), :])
                return rsb

            # ---- phase C: gather, out-proj, residual + LayerNorm, store
            def phase_c(w, rsb):
                import os as _os
                if _os.environ.get("KD_NO_C"):
                    return
                ctxF = ctxpool.tile([128, 1024], F16, tag="ctxF")
                nc.sync.dma_start(
                    ctxF[:].rearrange("p (c s) -> p c s", c=NC),
                    a2a_out[w][:].rearrange("c p s -> p c s"),
                )
                if _dump:
                    nc.sync.dma_start(cfo[w], ctxF[:])
                osb = lpool.tile([128, D], F32, tag="osb")
                acc_s = spool.tile([128, 1], F32, tag="acc_s")
                for n in range(2):
                    op = psS.tile([128, 512], F32, tag="sc", name="op")
                    for k in range(8):
                        nc.tensor.matmul(
                            op[:],
                            ctxF[:, 128 * k : 128 * (k + 1)],
                            won[n, k][:],
                            start=(k == 0),
                            stop=(k == 7),
                        )
                    nc.vector.tensor_tensor(
                        out=osb[:, 512 * n : 512 * (n + 1)],
                        in0=op[:],
                        in1=rsb[:, 512 * n : 512 * (n + 1)],
                        op=ADD,
                    )
                if _dump:
                    nc.sync.dma_start(oso[w], osb[:])
                nc.vector.tensor_reduce(acc_s[:], osb[:], mybir.AxisListType.X, ADD)
                scr = lpool.tile([128, D], F32, tag="scr")
                nc.vector.tensor_tensor(out=scr[:], in0=osb[:], in1=osb[:], op=MULT)
                acc_q = spool.tile([128, 1], F32, tag="acc_q")
                nc.vector.tensor_reduce(acc_q[:], scr[:], mybir.AxisListType.X, ADD)
                mean = spool.tile([128, 1], F32, tag="mean")
                nc.vector.tensor_scalar(mean[:], acc_s[:], 1.0 / D, None, MULT)
                msq = spool.tile([128, 1], F32, tag="msq")
                nc.vector.tensor_scalar(msq[:], acc_q[:], 1.0 / D, EPS, MULT, ADD)
                m2 = spool.tile([128, 1], F32, tag="m2")
                nc.vector.tensor_tensor(out=m2[:], in0=mean[:], in1=mean[:], op=MULT)
                var = spool.tile([128, 1], F32, tag="var")
                nc.vector.tensor_tensor(out=var[:], in0=msq[:], in1=m2[:], op=SUB)
                # rstd = 1/sqrt(var): magic seed + 2 Newton steps (all DVE,
                # keeps the ACT engine on the Exp table)
                ish = spool.tile([128, 1], I32, tag="ish")
                nc.vector.tensor_scalar(ish[:], var[:].bitcast(I32), 1, None, SHR)
                noti = spool.tile([128, 1], I32, tag="noti")
                nc.vector.tensor_scalar(noti[:], ish[:], -1, None, XOR)
                seed = spool.tile([128, 1], I32, tag="seed")
                nc.vector.tensor_scalar(seed[:], noti[:], RSQRT_MAGIC + 1, None, ADD)
                y = seed[:].bitcast(F32)
                for it in range(2):
                    a_ = spool.tile([128, 1], F32, tag=f"nra{it}", name=f"nra{it}")
                    nc.vector.tensor_tensor(out=a_[:], in0=y, in1=y, op=MULT)
                    b_ = spool.tile([128, 1], F32, tag=f"nrb{it}", name=f"nrb{it}")
                    nc.vector.tensor_tensor(out=b_[:], in0=a_[:], in1=var[:], op=MULT)
                    c_ = spool.tile([128, 1], F32, tag=f"nrc{it}", name=f"nrc{it}")
                    nc.vector.tensor_scalar(c_[:], b_[:], -0.5, 1.5, MULT, ADD)
                    y2 = spool.tile([128, 1], F32, tag=f"nry{it}", name=f"nry{it}")
                    nc.vector.tensor_tensor(out=y2[:], in0=y, in1=c_[:], op=MULT)
                    y = y2[:]
                mr = spool.tile([128, 1], F32, tag="mr")
                nc.vector.tensor_tensor(out=mr[:], in0=mean[:], in1=y, op=MULT)
                negmr = spool.tile([128, 1], F32, tag="negmr")
                nc.vector.tensor_scalar(negmr[:], mr[:], -1.0, None, MULT)
                onrm = lpool.tile([128, D], F32, tag="onrm")
                nc.scalar.activation(onrm[:], osb[:], IDENT, bias=negmr[:], scale=y)
                ow = lpool.tile([128, D], F32, tag="ow")
                nc.gpsimd.tensor_tensor(out=ow[:], in0=onrm[:], in1=lnw[:], op=MULT)
                ofin = lpool.tile([128, D], F32, tag="ofin")
                nc.gpsimd.tensor_tensor(out=ofin[:], in0=ow[:], in1=lnb[:], op=ADD)
                nc.sync.dma_start(outd[128 * w : 128 * (w + 1), :], ofin[:])

            # ---- schedule: interleave so exp/collectives overlap PE work and
            # each window's out-proj hides behind the next window's attention
            phase_a(0, 4)
            rsb0 = phase_b(0)
            rsb1 = phase_b(1)
            phase_c(0, rsb0)
            phase_a(4, 8)
            rsb2 = phase_b(2)
            phase_c(1, rsb1)
            rsb3 = phase_b(3)
            phase_c(2, rsb2)
            phase_c(3, rsb3)
            if _dump:
                nc.sync.dma_start(qSo[:], qS[:])
                nc.sync.dma_start(kSo[:], kS[:])
                nc.sync.dma_start(
                    vao[:], vaug[:].rearrange("p a b c -> p (a b c)")
                )

    nc.finalize()
    return nc


def _rot_matrix():
    # rotate_half as a left-multiply: out = R @ q for each 64-dim head block
    R = np.zeros((128, 128), np.float32)
    for h in range(HPC):
        o = HD * h
        for j in range(32):
            R[o + j, o + 32 + j] = -1.0
            R[o + 32 + j, o + j] = 1.0
    return np.ascontiguousarray(R.T.astype(np.float16))


def kernel(hidden_states, cos, sin, Wq, bq, Wk, bk, Wv, bv, Wo, bo, ln_w, ln_b):
    global LAST_RESULTS
    hs = np.ascontiguousarray(np.asarray(hidden_states, np.float32).reshape(NSEQ, D))
    cos = np.asarray(cos, np.float32)
    sin = np.asarray(sin, np.float32)
    Wq = np.asarray(Wq, np.float32)
    bq = np.asarray(bq, np.float32)
    Wk = np.asarray(Wk, np.float32)
    bk = np.asarray(bk, np.float32)
    Wv = np.asarray(Wv, np.float32)
    bv = np.asarray(bv, np.float32)
    Wo = np.asarray(Wo, np.float32)
    bo = np.asarray(bo, np.float32)
    ln_w = np.asarray(ln_w, np.float32)
    ln_b = np.asarray(ln_b, np.float32)

    xT = np.ascontiguousarray(hs.T.astype(np.float16))
    cosT = cos.T
    sinT = sin.T
    cs2 = np.ascontiguousarray(
        np.concatenate([cosT, cosT], axis=0).astype(np.float16)
    )
    sn2 = np.ascontiguousarray(
        np.concatenate([sinT, sinT], axis=0).astype(np.float16)
    )
    lnw_t = np.ascontiguousarray(np.tile(ln_w[None, :], (128, 1)))
    lnb_t = np.ascontiguousarray(np.tile(ln_b[None, :], (128, 1)))
    woT16 = np.ascontiguousarray(Wo.T.astype(np.float16))
    wq_s = (Wq / 64.0).T      # fold SCALING^2 into the q projection
    bq_s = bq / 64.0
    rt = _rot_matrix()

    # per-core residual rows: core p owns rows [1024*w + 128*p, +128) per window
    resid_full = hs + bo[None, :]

    in_maps = []
    for c in range(NC):
        fs = slice(F * c, F * (c + 1))
        rows = np.concatenate(
            [resid_full[1024 * w + 128 * c : 1024 * w + 128 * (c + 1)] for w in range(NW)],
            axis=0,
        )
        in_maps.append(
            {
                "xT": xT,
                "wq": np.ascontiguousarray(wq_s[:, fs].astype(np.float16)),
                "wk": np.ascontiguousarray(Wk.T[:, fs].astype(np.float16)),
                "wv": np.ascontiguousarray(Wv.T[:, fs].astype(np.float16)),
                "bq": np.ascontiguousarray(bq_s[fs, None]),
                "bk": np.ascontiguousarray(bk[fs, None]),
                "bv": np.ascontiguousarray(bv[fs, None]),
                "rt": rt,
                "woT": woT16,
                "cs2": cs2,
                "sn2": sn2,
                "resid": np.ascontiguousarray(rows),
                "lnw": lnw_t,
                "lnb": lnb_t,
            }
        )

    nc = _build()
    LAST_RESULTS = run_bass_kernel_spmd(nc, in_maps, core_ids=list(range(NC)))
    out = np.empty((NSEQ, D), np.float32)
    for c in range(NC):
        res = LAST_RESULTS.results[c]["out"]
        for w in range(NW):
            out[1024 * w + 128 * c : 1024 * w + 128 * (c + 1)] = res[
                128 * w : 128 * (w + 1)
            ]
    return out.reshape(B, S, D)


# revision 23
# speedup vs baseline: 1.3983x; 1.0231x over previous
"""NomicBertAttention on 8 Trainium2 NeuronCores.

Sharding: 8-way head tensor-parallelism (2 heads/core, both batches).
Per 1024-column window of the flattened (b,s) axis, an fp16 AllToAll
re-shards ctx^T by sequence rows (each core owns one 128-row block per
window), and the row-parallel out-proj + residual + LayerNorm for that
window is software-pipelined behind the next window's attention.

All PE matmuls run in fp16 (fp32 PSUM accumulation): projections with
the bias folded into the ACT-engine PSUM evacuation, RoPE rotate-half
via a constant +-1 rotation matrix on the PE, scores/ctx per head with
a ones-column in V giving the softmax denominator for free, and the
out-projection from the gathered fp16 ctx^T. Softmax denominators use
reciprocal_approx_fast; LayerNorm rstd uses a magic-constant rsqrt with
two Newton steps on the DVE so the ACT engine stays on the Exp table.
"""

import numpy as np
import concourse.bacc as bacc
import concourse.mybir as mybir
import concourse.tile as tile
from concourse.bass_utils import run_bass_kernel_spmd
from concourse.masks import make_identity

F32 = mybir.dt.float32
F16 = mybir.dt.float16
I32 = mybir.dt.int32
MULT = mybir.AluOpType.mult
ADD = mybir.AluOpType.add
SUB = mybir.AluOpType.subtract
XOR = mybir.AluOpType.bitwise_xor
SHR = mybir.AluOpType.arith_shift_right
BYPASS = mybir.AluOpType.bypass
EXP = mybir.ActivationFunctionType.Exp
IDENT = mybir.ActivationFunctionType.Identity

B, S, D, H, HD = 2, 2048, 1024, 16, 64
NC = 8
HPC = H // NC          # 2 heads per core
F = HPC * HD           # 128 projected features per core
NSEQ = B * S           # 4096 flattened rows
ROWS = NSEQ // NC      # 512 output rows per core (4 blocks of 128)
NW = 4                 # 1024-column windows
TB = S // 128          # 16 t-chunks per batch
EPS = 1e-12
RSQRT_MAGIC = 0x5F3759DF

LAST_RESULTS = None


def _build():
    nc = bacc.Bacc("TRN2", target_bir_lowering=False, debug=False, num_devices=NC)

    xT = nc.dram_tensor("xT", [D, NSEQ], F16, kind="ExternalInput")
    wq = nc.dram_tensor("wq", [D, F], F16, kind="ExternalInput")
    wk = nc.dram_tensor("wk", [D, F], F16, kind="ExternalInput")
    wv = nc.dram_tensor("wv", [D, F], F16, kind="ExternalInput")
    bqd = nc.dram_tensor("bq", [F, 1], F32, kind="ExternalInput")
    bkd = nc.dram_tensor("bk", [F, 1], F32, kind="ExternalInput")
    bvd = nc.dram_tensor("bv", [F, 1], F32, kind="ExternalInput")
    rtd = nc.dram_tensor("rt", [128, 128], F16, kind="ExternalInput")
    woT = nc.dram_tensor("woT", [D, D], F16, kind="ExternalInput")
    cs2d = nc.dram_tensor("cs2", [128, S], F16, kind="ExternalInput")
    sn2d = nc.dram_tensor("sn2", [128, S], F16, kind="ExternalInput")
    residd = nc.dram_tensor("resid", [ROWS, D], F32, kind="ExternalInput")
    lnwd = nc.dram_tensor("lnw", [128, D], F32, kind="ExternalInput")
    lnbd = nc.dram_tensor("lnb", [128, D], F32, kind="ExternalInput")
    outd = nc.dram_tensor("out", [ROWS, D], F32, kind="ExternalOutput")
    import os as _os0
    _dump = bool(_os0.environ.get("KD_DUMP"))
    if _dump:
        qSo = nc.dram_tensor("qSo", [128, NSEQ], F16, kind="ExternalOutput")
        kSo = nc.dram_tensor("kSo", [128, NSEQ], F16, kind="ExternalOutput")
        vao = nc.dram_tensor("vao", [128, 2 * TB * HPC * (HD + 1)], F16, kind="ExternalOutput")
        cto = nc.dram_tensor("cto", [NW, 128, 1024], F16, kind="ExternalOutput")
        cfo = nc.dram_tensor("cfo", [NW, 128, 1024], F16, kind="ExternalOutput")
        oso = nc.dram_tensor("oso", [NW, 128, D], F32, kind="ExternalOutput")

    with tile.TileContext(nc) as tc:
        with (
            tc.tile_pool(name="qk", bufs=1) as qkpool,
            tc.tile_pool(name="wpool", bufs=1) as wpool,
            tc.tile_pool(name="xpool", bufs=2) as xpool,
            tc.tile_pool(name="rope", bufs=2) as rpool,
            tc.tile_pool(name="exps", bufs=3) as epool,
            tc.tile_pool(name="ctxp", bufs=2) as ctxpool,
            tc.tile_pool(name="bcastp", bufs=2) as bpool,
            tc.tile_pool(name="small", bufs=4) as spool,
            tc.tile_pool(name="tail", bufs=2) as lpool,
            tc.tile_pool(name="psS", bufs=2, space="PSUM") as psS,
            tc.tile_pool(name="psC", bufs=4, space="PSUM") as psC,
            tc.tile_pool(name="dram", bufs=1, space="DRAM") as dpool,
        ):
            # ---- resident tensors
            qS = qkpool.tile([128, NSEQ], F16, tag="qS")
            kS = qkpool.tile([128, NSEQ], F16, tag="kS")
            # v natural + ones column: [t-part, tcg, head, 64+1]
            vaug = qkpool.tile([128, 2 * TB, HPC, HD + 1], F16, tag="vaug")
            nc.vector.memset(vaug[:, :, :, HD : HD + 1], 1.0)

            cs2 = wpool.tile([128, S], F16, tag="cs2")
            sn2 = wpool.tile([128, S], F16, tag="sn2")
            nc.sync.dma_start(cs2[:], cs2d[:])
            nc.sync.dma_start(sn2[:], sn2d[:])
            rt = wpool.tile([128, 128], F16, tag="rt")
            nc.sync.dma_start(rt[:], rtd[:])
            ident = wpool.tile([128, 128], F16, tag="ident")
            make_identity(nc, ident[:])
            wsb = {}
            bias_t = {}
            for name, dram_w, dram_b in (("q", wq, bqd), ("k", wk, bkd), ("v", wv, bvd)):
                wt = wpool.tile([128, D], F16, tag=f"w{name}", name=f"w{name}")
                for k in range(8):
                    nc.sync.dma_start(
                        wt[:, 128 * k : 128 * (k + 1)],
                        dram_w[128 * k : 128 * (k + 1), :],
                    )
                wsb[name] = wt
                bt = wpool.tile([128, 1], F32, tag=f"b{name}", name=f"b{name}")
                nc.sync.dma_start(bt[:], dram_b[:])
                bias_t[name] = bt
            won = {}
            for n in range(2):
                for k in range(8):
                    wot = wpool.tile([128, 512], F16, tag=f"wo{n}{k}", name=f"wo{n}{k}")
                    nc.sync.dma_start(
                        wot[:], woT[128 * k : 128 * (k + 1), 512 * n : 512 * (n + 1)]
                    )
                    won[n, k] = wot
            lnw = wpool.tile([128, D], F32, tag="lnw")
            lnb = wpool.tile([128, D], F32, tag="lnb")
            nc.sync.dma_start(lnw[:], lnwd[:])
            nc.sync.dma_start(lnb[:], lnbd[:])

            a2a_in = {}
            a2a_out = {}
            for w in range(NW):
                a2a_in[w] = dpool.tile([NC, 128, 128], F16, tag=f"a2a_in{w}", name=f"a2a_in{w}")
                a2a_out[w] = dpool.tile([NC, 128, 128], F16, tag=f"a2a_out{w}", name=f"a2a_out{w}")

            # warmup collective: absorbs the one-time CC ring setup (~30us)
            # while phase A computes, so the first real AllToAll is fast
            warm_in = dpool.tile([NC, 1, 4], F32, tag="warm_in")
            warm_out = dpool.tile([NC, 1, 4], F32, tag="warm_out")
            warm_src = spool.tile([1, 4 * NC], F32, tag="warm_src", bufs=1)
            nc.vector.memset(warm_src[:], 0.0)
            nc.sync.dma_start(warm_in[:].rearrange("c p s -> p (c s)"), warm_src[:])
            nc.gpsimd.collective_compute(
                "AllToAll",
                BYPASS,
                replica_groups=[list(range(NC))],
                ins=[warm_in[:]],
                outs=[warm_out[:]],
            )

            # ---- phase A: projections + RoPE + v transpose for groups [g0, g1)
            def phase_a(g0, g1):
                for g in range(g0, g1):
                    gs = 512 * g
                    cg = gs % S
                    xg = xpool.tile([128, 4096], F16, tag="xg")
                    for k in range(8):
                        nc.sync.dma_start(
                            xg[:, 512 * k : 512 * (k + 1)],
                            xT[128 * k : 128 * (k + 1), gs : gs + 512],
                        )
                    for name in ("q", "k", "v"):
                        pp = psC.tile([128, 512], F32, tag="cp", name="pp")
                        for k in range(8):
                            nc.tensor.matmul(
                                pp[:],
                                wsb[name][:, 128 * k : 128 * (k + 1)],
                                xg[:, 512 * k : 512 * (k + 1)],
                                start=(k == 0),
                                stop=(k == 7),
                            )
                        plain = rpool.tile([128, 512], F16, tag="plain")
                        nc.scalar.activation(plain[:], pp[:], IDENT, bias=bias_t[name][:])
                        if name in ("q", "k"):
                            rotp = psC.tile([128, 512], F32, tag="cp", name="rotp")
                            nc.tensor.matmul(rotp[:], rt[:], plain[:], start=True, stop=True)
                            tc_ = rpool.tile([128, 512], F16, tag="tc")
                            nc.vector.tensor_tensor(
                                out=tc_[:], in0=plain[:], in1=cs2[:, cg : cg + 512], op=MULT
                            )
                            ts_ = rpool.tile([128, 512], F16, tag="ts")
                            nc.vector.tensor_tensor(
                                out=ts_[:], in0=rotp[:], in1=sn2[:, cg : cg + 512], op=MULT
                            )
                            dst = qS if name == "q" else kS
                            nc.gpsimd.tensor_tensor(
                                out=dst[:, gs : gs + 512], in0=tc_[:], in1=ts_[:], op=ADD
                            )
                        else:
                            for sub in range(4):
                                trp = psC.tile([128, 128], F16, tag="cp", name="trp")
                                nc.tensor.transpose(
                                    trp[:], plain[:, 128 * sub : 128 * (sub + 1)], ident[:]
                                )
                                tcg = 4 * g + sub
                                nc.vector.tensor_copy(
                                    vaug[:, tcg, :, 0:HD],
                                    trp[:].rearrange("p (h d) -> p h d", h=HPC),
                                )

            # ---- phase B: attention for window w (1024 s-columns), then
            # normalize + AllToAll launch (out-proj deferred to phase_c)
            def phase_b(w):
                b = w // 2
                sw = 1024 * w
                cps = {}
                for h in range(HPC):
                    for half in range(2):
                        cps[h, half] = psC.tile(
                            [HD + 1, 512], F32, tag="cp", name=f"cps_{h}_{half}"
                        )
                for tcl in range(TB):
                    tg = S * b + 128 * tcl
                    tcg = TB * b + tcl
                    for h in range(HPC):
                        hs_, he = HD * h, HD * (h + 1)
                        sc = psS.tile([128, 1024], F32, tag="sc")
                        for half in range(2):
                            s0 = sw + 512 * half
                            nc.tensor.matmul(
                                sc[:, 512 * half : 512 * (half + 1)],
                                kS[hs_:he, tg : tg + 128],
                                qS[hs_:he, s0 : s0 + 512],
                                start=True,
                                stop=True,
                            )
                        ex = epool.tile([128, 1024], F16, tag="ex")
                        nc.scalar.activation(ex[:], sc[:], EXP)
                        for half in range(2):
                            nc.tensor.matmul(
                                cps[h, half][:],
                                vaug[:, tcg, h, :],
                                ex[:, 512 * half : 512 * (half + 1)],
                                start=(tcl == 0),
                                stop=(tcl == TB - 1),
                            )
                # normalize ctx by the denominator row and ship via AllToAll
                ctile = ctxpool.tile([128, 1024], F16, tag="ctile")
                for h in range(HPC):
                    for half in range(2):
                        dsb = spool.tile([1, 512], F32, tag="dsb")
                        nc.vector.tensor_copy(dsb[:], cps[h, half][HD : HD + 1, :])
                        rden = spool.tile([1, 512], F32, tag="rden")
                        nc.vector.reciprocal_approx_fast(rden[:], dsb[:])
                        bc = bpool.tile([HD, 512], F32, tag="bc")
                        nc.gpsimd.partition_broadcast(bc[:], rden[:])
                        nc.vector.tensor_tensor(
                            out=ctile[HD * h : HD * (h + 1), 512 * half : 512 * (half + 1)],
                            in0=cps[h, half][0:HD, :],
                            in1=bc[:],
                            op=MULT,
                        )
                if _dump:
                    nc.sync.dma_start(cto[w], ctile[:])
                nc.sync.dma_start(
                    a2a_in[w][:].rearrange("c p s -> p c s"),
                    ctile[:].rearrange("p (c s) -> p c s", c=NC),
                )
                import os as _os
                if _os.environ.get("KD_NO_CC"):
                    nc.sync.dma_start(a2a_out[w][:], a2a_in[w][:])
                else:
                    nc.gpsimd.collective_compute(
                        "AllToAll",
                        BYPASS,
                        replica_groups=[list(range(NC))],
                        ins=[a2a_in[w][:]],
                        outs=[a2a_out[w][:]],
                    )
                rsb = lpool.tile([128, D], F32, tag="rsb", name=f"rsb{w}")
                nc.sync.dma_start(rsb[:], residd[128 * w : 128 * (w + 1), :])
                return rsb

            # ---- phase C: gather, out-proj, residual + LayerNorm, store
            def phase_c(w, rsb):
                import os as _os
                if _os.environ.get("KD_NO_C"):
                    return
                # gather + output DMAs ride the ACT hwdge queue so they don't
                # serialize behind next-window a2a_in triggers on the SP queue
                ctxF = ctxpool.tile([128, 1024], F16, tag="ctxF")
                nc.scalar.dma_start(
                    ctxF[:].rearrange("p (c s) -> p c s", c=NC),
                    a2a_out[w][:].rearrange("c p s -> p c s"),
                )
                if _dump:
                    nc.sync.dma_start(cfo[w], ctxF[:])
                osb = lpool.tile([128, D], F32, tag="osb")
                acc_s = spool.tile([128, 1], F32, tag="acc_s")
                for n in range(2):
                    op = psS.tile([128, 512], F32, tag="sc", name="op")
                    for k in range(8):
                        nc.tensor.matmul(
                            op[:],
                            ctxF[:, 128 * k : 128 * (k + 1)],
                            won[n, k][:],
                            start=(k == 0),
                            stop=(k == 7),
                        )
                    nc.vector.tensor_tensor(
                        out=osb[:, 512 * n : 512 * (n + 1)],
                        in0=op[:],
                        in1=rsb[:, 512 * n : 512 * (n + 1)],
                        op=ADD,
                    )
                if _dump:
                    nc.sync.dma_start(oso[w], osb[:])
                # mean+var in two DVE ops via the BN statistics instructions
                stats = spool.tile([128, 12], F32, tag="stats")
                for n in range(2):
                    nc.vector.bn_stats(
                        stats[:, 6 * n : 6 * (n + 1)],
                        osb[:, 512 * n : 512 * (n + 1)],
                    )
                mv = spool.tile([128, 2], F32, tag="mv")
                nc.vector.bn_aggr(mv[:], stats[:])
                mean = mv[:, 0:1]
                var = mv[:, 1:2]
                # rstd = 1/sqrt(var): magic seed + 2 Newton steps (all DVE,
                # keeps the ACT engine on the Exp table)
                ish = spool.tile([128, 1], I32, tag="ish")
                nc.vector.tensor_scalar(ish[:], var.bitcast(I32), 1, None, SHR)
                noti = spool.tile([128, 1], I32, tag="noti")
                nc.vector.tensor_scalar(noti[:], ish[:], -1, None, XOR)
                seed = spool.tile([128, 1], I32, tag="seed")
                nc.vector.tensor_scalar(seed[:], noti[:], RSQRT_MAGIC + 1, None, ADD)
                y = seed[:].bitcast(F32)
                for it in range(2):
                    a_ = spool.tile([128, 1], F32, tag=f"nra{it}", name=f"nra{it}")
                    nc.vector.tensor_tensor(out=a_[:], in0=y, in1=y, op=MULT)
                    b_ = spool.tile([128, 1], F32, tag=f"nrb{it}", name=f"nrb{it}")
                    nc.vector.tensor_tensor(out=b_[:], in0=a_[:], in1=var, op=MULT)
                    c_ = spool.tile([128, 1], F32, tag=f"nrc{it}", name=f"nrc{it}")
                    nc.vector.tensor_scalar(c_[:], b_[:], -0.5, 1.5, MULT, ADD)
                    y2 = spool.tile([128, 1], F32, tag=f"nry{it}", name=f"nry{it}")
                    nc.vector.tensor_tensor(out=y2[:], in0=y, in1=c_[:], op=MULT)
                    y = y2[:]
                mr = spool.tile([128, 1], F32, tag="mr")
                nc.vector.tensor_tensor(out=mr[:], in0=mean, in1=y, op=MULT)
                negmr = spool.tile([128, 1], F32, tag="negmr")
                nc.vector.tensor_scalar(negmr[:], mr[:], -1.0, None, MULT)
                onrm = lpool.tile([128, D], F32, tag="onrm")
                nc.scalar.activation(onrm[:], osb[:], IDENT, bias=negmr[:], scale=y)
                ow = lpool.tile([128, D], F32, tag="ow")
                nc.vector.tensor_tensor(out=ow[:], in0=onrm[:], in1=lnw[:], op=MULT)
                ofin = lpool.tile([128, D], F32, tag="ofin")
                nc.vector.tensor_tensor(out=ofin[:], in0=ow[:], in1=lnb[:], op=ADD)
                nc.scalar.dma_start(outd[128 * w : 128 * (w + 1), :], ofin[:])

            # ---- schedule: interleave so exp/collectives overlap PE work and
            # each window's out-proj hides behind the next window's attention
            phase_a(0, 4)
            rsb0 = phase_b(0)
            rsb1 = phase_b(1)
            phase_a(4, 8)
            phase_c(0, rsb0)
            rsb2 = phase_b(2)
            phase_c(1, rsb1)
            rsb3 = phase_b(3)
            phase_c(2, rsb2)
            phase_c(3, rsb3)
            if _dump:
                nc.sync.dma_start(qSo[:], qS[:])
                nc.sync.dma_start(kSo[:], kS[:])
                nc.sync.dma_start(
                    vao[:], vaug[:].rearrange("p a b c -> p (a b c)")
                )

    nc.finalize()
    return nc


def _rot_matrix():
    # rotate_half as a left-multiply: out = R @ q for each 64-dim head block
    R = np.zeros((128, 128), np.float32)
    for h in range(HPC):
        o = HD * h
        for j in range(32):
            R[o + j, o + 32 + j] = -1.0
            R[o + 32 + j, o + j] = 1.0
    return np.ascontiguousarray(R.T.astype(np.float16))


def kernel(hidden_states, cos, sin, Wq, bq, Wk, bk, Wv, bv, Wo, bo, ln_w, ln_b):
    global LAST_RESULTS
    hs = np.ascontiguousarray(np.asarray(hidden_states, np.float32).reshape(NSEQ, D))
    cos = np.asarray(cos, np.float32)
    sin = np.asarray(sin, np.float32)
    Wq = np.asarray(Wq, np.float32)
    bq = np.asarray(bq, np.float32)
    Wk = np.asarray(Wk, np.float32)
    bk = np.asarray(bk, np.float32)
    Wv = np.asarray(Wv, np.float32)
    bv = np.asarray(bv, np.float32)
    Wo = np.asarray(Wo, np.float32)
    bo = np.asarray(bo, np.float32)
    ln_w = np.asarray(ln_w, np.float32)
    ln_b = np.asarray(ln_b, np.float32)

    xT = np.ascontiguousarray(hs.T.astype(np.float16))
    cosT = cos.T
    sinT = sin.T
    cs2 = np.ascontiguousarray(
        np.concatenate([cosT, cosT], axis=0).astype(np.float16)
    )
    sn2 = np.ascontiguousarray(
        np.concatenate([sinT, sinT], axis=0).astype(np.float16)
    )
    lnw_t = np.ascontiguousarray(np.tile(ln_w[None, :], (128, 1)))
    lnb_t = np.ascontiguousarray(np.tile(ln_b[None, :], (128, 1)))
    woT16 = np.ascontiguousarray(Wo.T.astype(np.float16))
    wq_s = (Wq / 64.0).T      # fold SCALING^2 into the q projection
    bq_s = bq / 64.0
    rt = _rot_matrix()

    # per-core residual rows: core p owns rows [1024*w + 128*p, +128) per window
    resid_full = hs + bo[None, :]

    in_maps = []
    for c in range(NC):
        fs = slice(F * c, F * (c + 1))
        rows = np.concatenate(
            [resid_full[1024 * w + 128 * c : 1024 * w + 128 * (c + 1)] for w in range(NW)],
            axis=0,
        )
        in_maps.append(
            {
                "xT": xT,
                "wq": np.ascontiguousarray(wq_s[:, fs].astype(np.float16)),
                "wk": np.ascontiguousarray(Wk.T[:, fs].astype(np.float16)),
                "wv": np.ascontiguousarray(Wv.T[:, fs].astype(np.float16)),
                "bq": np.ascontiguousarray(bq_s[fs, None]),
                "bk": np.ascontiguousarray(bk[fs, None]),
                "bv": np.ascontiguousarray(bv[fs, None]),
                "rt": rt,
                "woT": woT16,
                "cs2": cs2,
                "sn2": sn2,
                "resid": np.ascontiguousarray(rows),
                "lnw": lnw_t,
                "lnb": lnb_t,
            }
        )

    nc = _build()
    LAST_RESULTS = run_bass_kernel_spmd(nc, in_maps, core_ids=list(range(NC)))
    out = np.empty((NSEQ, D), np.float32)
    for c in range(NC):
        res = LAST_RESULTS.results[c]["out"]
        for w in range(NW):
            out[1024 * w + 128 * c : 1024 * w + 128 * (c + 1)] = res[
                128 * w : 128 * (w + 1)
            ]
    return out.reshape(B, S, D)


# revision 26
# speedup vs baseline: 1.5880x; 1.1357x over previous
"""NomicBertAttention on 8 Trainium2 NeuronCores.

Sharding: 8-way head tensor-parallelism (2 heads/core, both batches).
Per 1024-column window of the flattened (b,s) axis, an fp16 AllToAll
re-shards ctx^T by sequence rows (each core owns one 128-row block per
window), and the row-parallel out-proj + residual + LayerNorm for that
window is software-pipelined behind the next window's attention.

Attention matmuls run in fp8e4m3 DoubleRow mode (0.5 cycles/row, fp32
PSUM): projections contract K=256 per weight load, scores contract the
64-dim head as 2x32 (head-dim halves interleaved in the free dim via a
host-side feature permutation), and ctx contracts 2 t-chunks (K=2x128)
with a ones-column in V producing the softmax denominator. Scale
folding keeps fp8 in its normal range: W_{q,k,v} are scaled x16
host-side, the exp activation applies 1/(64*256) (RoPE q,k are x16 and
the reference's double 1/8 scaling), and Wo carries the remaining 1/16.
The residual/LayerNorm path stays fp32, so fp8 noise only touches the
small attention contribution. RoPE rotate-half rides a constant +-1
matrix on the PE; LayerNorm stats use bn_stats/bn_aggr and a
magic-constant rsqrt on the DVE so the ACT engine stays on the Exp
table (the last window uses ACT Sqrt since no exp follows it).
"""

import numpy as np
import ml_dtypes
import concourse.bacc as bacc
import concourse.mybir as mybir
import concourse.tile as tile
from concourse.bass_utils import run_bass_kernel_spmd
from concourse.masks import make_identity

F32 = mybir.dt.float32
F16 = mybir.dt.float16
F8 = mybir.dt.float8e4
I32 = mybir.dt.int32
DR = mybir.MatmulPerfMode.DoubleRow
MULT = mybir.AluOpType.mult
ADD = mybir.AluOpType.add
SUB = mybir.AluOpType.subtract
XOR = mybir.AluOpType.bitwise_xor
SHR = mybir.AluOpType.arith_shift_right
BYPASS = mybir.AluOpType.bypass
EXP = mybir.ActivationFunctionType.Exp
IDENT = mybir.ActivationFunctionType.Identity
SQRT = mybir.ActivationFunctionType.Sqrt

B, S, D, H, HD = 2, 2048, 1024, 16, 64
NC = 8
HPC = H // NC          # 2 heads per core
F = HPC * HD           # 128 projected features per core
NSEQ = B * S           # 4096 flattened rows
ROWS = NSEQ // NC      # 512 output rows per core (4 blocks of 128)
NW = 4                 # 1024-column windows
TB = S // 128          # 16 t-chunks per batch
NPAIR = TB // 2        # 8 t-chunk pairs per batch
MV = 80                # ctx stationary free (64 v + 1 ones + 15 pad; mult of 16)
WSCALE = 16.0          # fp8 range scaling folded into Wq/Wk/Wv
EXP_SCALE = 1.0 / (64.0 * WSCALE * WSCALE)
EPS = 1e-12
RSQRT_MAGIC = 0x5F3759DF

LAST_RESULTS = None


def _build():
    nc = bacc.Bacc("TRN2", target_bir_lowering=False, debug=False, num_devices=NC)

    xT = nc.dram_tensor("xT", [D, NSEQ], F8, kind="ExternalInput")
    wq = nc.dram_tensor("wq", [128, D], F8, kind="ExternalInput")
    wk = nc.dram_tensor("wk", [128, D], F8, kind="ExternalInput")
    wv = nc.dram_tensor("wv", [128, D], F8, kind="ExternalInput")
    bqd = nc.dram_tensor("bq", [F, 1], F32, kind="ExternalInput")
    bkd = nc.dram_tensor("bk", [F, 1], F32, kind="ExternalInput")
    bvd = nc.dram_tensor("bv", [F, 1], F32, kind="ExternalInput")
    rtd = nc.dram_tensor("rt", [128, 128], F16, kind="ExternalInput")
    woT = nc.dram_tensor("woT", [D, D], F16, kind="ExternalInput")
    cs2d = nc.dram_tensor("cs2", [128, S], F16, kind="ExternalInput")
    sn2d = nc.dram_tensor("sn2", [128, S], F16, kind="ExternalInput")
    residd = nc.dram_tensor("resid", [ROWS, D], F32, kind="ExternalInput")
    lnwd = nc.dram_tensor("lnw", [128, D], F32, kind="ExternalInput")
    lnbd = nc.dram_tensor("lnb", [128, D], F32, kind="ExternalInput")
    outd = nc.dram_tensor("out", [ROWS, D], F32, kind="ExternalOutput")
    import os as _os0
    _dump = bool(_os0.environ.get("KD_DUMP"))
    if _dump:
        cto = nc.dram_tensor("cto", [NW, 128, 1024], F16, kind="ExternalOutput")
        cfo = nc.dram_tensor("cfo", [NW, 128, 1024], F16, kind="ExternalOutput")
        oso = nc.dram_tensor("oso", [NW, 128, D], F32, kind="ExternalOutput")

    with tile.TileContext(nc) as tc:
        with (
            tc.tile_pool(name="qk", bufs=1) as qkpool,
            tc.tile_pool(name="wpool", bufs=1) as wpool,
            tc.tile_pool(name="xpool", bufs=2) as xpool,
            tc.tile_pool(name="rope", bufs=2) as rpool,
            tc.tile_pool(name="exps", bufs=3) as epool,
            tc.tile_pool(name="ctxp", bufs=2) as ctxpool,
            tc.tile_pool(name="bcastp", bufs=2) as bpool,
            tc.tile_pool(name="small", bufs=4) as spool,
            tc.tile_pool(name="tail", bufs=2) as lpool,
            tc.tile_pool(name="psS", bufs=2, space="PSUM") as psS,
            tc.tile_pool(name="psC", bufs=4, space="PSUM") as psC,
            tc.tile_pool(name="dram", bufs=1, space="DRAM") as dpool,
        ):
            # ---- resident tensors
            # q/k RoPE'd fp8: [32h+dlo (64 parts), d-half, pos]
            qS8 = qkpool.tile([64, 2, NSEQ], F8, tag="qS8")
            kS8 = qkpool.tile([64, 2, NSEQ], F8, tag="kS8")
            # v natural fp8 + ones col: [t-part, pair, half-of-pair, head, MV]
            vaug = qkpool.tile([128, 2 * NPAIR, 2, HPC, MV], F8, tag="vaug")
            nc.vector.memset(vaug[:, :, :, :, HD : HD + 1], 1.0)
            nc.vector.memset(vaug[:, :, :, :, HD + 1 : MV], 0.0)

            a2a_in = {}
            a2a_out = {}
            for w in range(NW):
                a2a_in[w] = dpool.tile([NC, 128, 128], F16, tag=f"a2a_in{w}", name=f"a2a_in{w}")
                a2a_out[w] = dpool.tile([NC, 128, 128], F16, tag=f"a2a_out{w}", name=f"a2a_out{w}")

            # warmup collective: absorbs the one-time CC ring setup while
            # phase A computes, so the first real AllToAll is fast
            warm_in = dpool.tile([NC, 1, 4], F32, tag="warm_in")
            warm_out = dpool.tile([NC, 1, 4], F32, tag="warm_out")
            warm_src = spool.tile([1, 4 * NC], F32, tag="warm_src", bufs=1)
            nc.vector.memset(warm_src[:], 0.0)
            nc.sync.dma_start(warm_in[:].rearrange("c p s -> p (c s)"), warm_src[:])
            nc.gpsimd.collective_compute(
                "AllToAll",
                BYPASS,
                replica_groups=[list(range(NC))],
                ins=[warm_in[:]],
                outs=[warm_out[:]],
            )

            # early weights on the SP queue (needed by phase A)
            cs2 = wpool.tile([128, S], F16, tag="cs2")
            sn2 = wpool.tile([128, S], F16, tag="sn2")
            nc.sync.dma_start(cs2[:], cs2d[:])
            nc.sync.dma_start(sn2[:], sn2d[:])
            rt = wpool.tile([128, 128], F16, tag="rt")
            nc.sync.dma_start(rt[:], rtd[:])
            ident = wpool.tile([128, 128], F16, tag="ident")
            make_identity(nc, ident[:])
            wsb = {}
            bias_t = {}
            for name, dram_w, dram_b in (("q", wq, bqd), ("k", wk, bkd), ("v", wv, bvd)):
                wt = wpool.tile([128, 4, 2, 128], F8, tag=f"w{name}", name=f"w{name}")
                nc.sync.dma_start(wt[:].rearrange("p a b c -> p (a b c)"), dram_w[:])
                wsb[name] = wt
                bt = wpool.tile([128, 1], F32, tag=f"b{name}", name=f"b{name}")
                nc.sync.dma_start(bt[:, 0:1], dram_b[:])
                bias_t[name] = bt
            # late weights on the ACT hwdge queue (needed from phase C on)
            won = {}
            for n in range(2):
                for k in range(8):
                    wot = wpool.tile([128, 512], F16, tag=f"wo{n}{k}", name=f"wo{n}{k}")
                    nc.scalar.dma_start(
                        wot[:], woT[128 * k : 128 * (k + 1), 512 * n : 512 * (n + 1)]
                    )
                    won[n, k] = wot
            lnw = wpool.tile([128, D], F32, tag="lnw")
            lnb = wpool.tile([128, D], F32, tag="lnb")
            nc.scalar.dma_start(lnw[:], lnwd[:])
            nc.scalar.dma_start(lnb[:], lnbd[:])

            # ---- phase A: fp8 projections + RoPE + v transpose
            def phase_a(g0, g1):
                for g in range(g0, g1):
                    gs = 512 * g
                    cg = gs % S
                    xg = xpool.tile([128, 4, 2, 512], F8, tag="xg")
                    for cp in range(4):
                        for i in range(2):
                            nc.sync.dma_start(
                                xg[:, cp, i, :],
                                xT[256 * cp + 128 * i : 256 * cp + 128 * (i + 1), gs : gs + 512],
                            )
                    for name in ("q", "k", "v"):
                        pp = psC.tile([128, 512], F32, tag="cp", name="pp")
                        for cp in range(4):
                            nc.tensor.matmul(
                                pp[:],
                                wsb[name][:, cp, :, :],
                                xg[:, cp, :, :],
                                start=(cp == 0),
                                stop=(cp == 3),
                                perf_mode=DR,
                            )
                        plain = rpool.tile([128, 512], F16, tag="plain")
                        nc.scalar.activation(plain[:], pp[:], IDENT, bias=bias_t[name][:])
                        if name in ("q", "k"):
                            rotp = psC.tile([128, 512], F32, tag="cp", name="rotp")
                            nc.tensor.matmul(rotp[:], rt[:], plain[:], start=True, stop=True)
                            tc_ = rpool.tile([128, 512], F16, tag="tc")
                            nc.vector.tensor_tensor(
                                out=tc_[:], in0=plain[:], in1=cs2[:, cg : cg + 512], op=MULT
                            )
                            ts_ = rpool.tile([128, 512], F16, tag="ts")
                            nc.vector.tensor_tensor(
                                out=ts_[:], in0=rotp[:], in1=sn2[:, cg : cg + 512], op=MULT
                            )
                            dst = qS8 if name == "q" else kS8
                            for i in range(2):
                                nc.gpsimd.tensor_tensor(
                                    out=dst[0:64, i, gs : gs + 512],
                                    in0=tc_[64 * i : 64 * (i + 1), :],
                                    in1=ts_[64 * i : 64 * (i + 1), :],
                                    op=ADD,
                                )
                        else:
                            for sub in range(4):
                                trp = psC.tile([128, 128], F16, tag="cp", name="trp")
                                nc.tensor.transpose(
                                    trp[:], plain[:, 128 * sub : 128 * (sub + 1)], ident[:]
                                )
                                tcg = 4 * g + sub
                                nc.vector.tensor_copy(
                                    vaug[:, tcg // 2, tcg % 2, :, 0:HD],
                                    trp[:].rearrange("p (h d) -> p h d", h=HPC),
                                )

            # ---- phase B: attention for window w, then normalize + AllToAll
            def phase_b(w):
                b = w // 2
                sw = 1024 * w
                cps = {}
                for h in range(HPC):
                    for half in range(2):
                        cps[h, half] = psC.tile(
                            [MV, 512], F32, tag="cp", name=f"cps_{h}_{half}"
                        )
                for pg in range(NPAIR):
                    ex8 = {}
                    for h in range(HPC):
                        ex8[h] = epool.tile([128, 2, 1024], F8, tag="ex", name=f"ex{h}")
                    for i in range(2):
                        tcl = 2 * pg + i
                        tg = S * b + 128 * tcl
                        for h in range(HPC):
                            sc = psS.tile([128, 1024], F32, tag="sc")
                            for half in range(2):
                                s0 = sw + 512 * half
                                nc.tensor.matmul(
                                    sc[:, 512 * half : 512 * (half + 1)],
                                    kS8[32 * h : 32 * (h + 1), :, tg : tg + 128],
                                    qS8[32 * h : 32 * (h + 1), :, s0 : s0 + 512],
                                    start=True,
                                    stop=True,
                                    perf_mode=DR,
                                )
                            nc.scalar.activation(
                                ex8[h][:, i, :], sc[:], EXP, scale=EXP_SCALE
                            )
                    pgg = NPAIR * b + pg
                    for h in range(HPC):
                        for half in range(2):
                            nc.tensor.matmul(
                                cps[h, half][:],
                                vaug[:, pgg, :, h, :],
                                ex8[h][:, :, 512 * half : 512 * (half + 1)],
                                start=(pg == 0),
                                stop=(pg == NPAIR - 1),
                                perf_mode=DR,
                            )
                # normalize ctx by the denominator row and ship via AllToAll
                ctile = ctxpool.tile([128, 1024], F16, tag="ctile")
                for h in range(HPC):
                    for half in range(2):
                        dsb = spool.tile([1, 512], F32, tag="dsb")
                        nc.vector.tensor_copy(dsb[:], cps[h, half][HD : HD + 1, :])
                        rden = spool.tile([1, 512], F32, tag="rden")
                        nc.vector.reciprocal_approx_fast(rden[:], dsb[:])
                        bc = bpool.tile([HD, 512], F32, tag="bc")
                        nc.gpsimd.partition_broadcast(bc[:], rden[:])
                        nc.vector.tensor_tensor(
                            out=ctile[HD * h : HD * (h + 1), 512 * half : 512 * (half + 1)],
                            in0=cps[h, half][0:HD, :],
                            in1=bc[:],
                            op=MULT,
                        )
                if _dump:
                    nc.sync.dma_start(cto[w], ctile[:])
                nc.sync.dma_start(
                    a2a_in[w][:].rearrange("c p s -> p c s"),
                    ctile[:].rearrange("p (c s) -> p c s", c=NC),
                )
                nc.gpsimd.collective_compute(
                    "AllToAll",
                    BYPASS,
                    replica_groups=[list(range(NC))],
                    ins=[a2a_in[w][:]],
                    outs=[a2a_out[w][:]],
                )
                rsb = lpool.tile([128, D], F32, tag="rsb", name=f"rsb{w}")
                nc.sync.dma_start(rsb[:], residd[128 * w : 128 * (w + 1), :])
                return rsb

            # ---- phase C: gather, out-proj, residual + LayerNorm, store
            def phase_c(w, rsb):
                # gather + output DMAs ride the ACT hwdge queue so they don't
                # serialize behind next-window a2a_in triggers on the SP queue
                ctxF = ctxpool.tile([128, 1024], F16, tag="ctxF")
                nc.scalar.dma_start(
                    ctxF[:].rearrange("p (c s) -> p c s", c=NC),
                    a2a_out[w][:].rearrange("c p s -> p c s"),
                )
                if _dump:
                    nc.sync.dma_start(cfo[w], ctxF[:])
                osb = lpool.tile([128, D], F32, tag="osb")
                for n in range(2):
                    op = psS.tile([128, 512], F32, tag="sc", name="op")
                    for k in range(8):
                        nc.tensor.matmul(
                            op[:],
                            ctxF[:, 128 * k : 128 * (k + 1)],
                            won[n, k][:],
                            start=(k == 0),
                            stop=(k == 7),
                        )
                    nc.vector.tensor_tensor(
                        out=osb[:, 512 * n : 512 * (n + 1)],
                        in0=op[:],
                        in1=rsb[:, 512 * n : 512 * (n + 1)],
                        op=ADD,
                    )
                if _dump:
                    nc.sync.dma_start(oso[w], osb[:])
                # mean+var in three DVE ops via the BN statistics instructions
                stats = spool.tile([128, 12], F32, tag="stats")
                for n in range(2):
                    nc.vector.bn_stats(
                        stats[:, 6 * n : 6 * (n + 1)],
                        osb[:, 512 * n : 512 * (n + 1)],
                    )
                mv = spool.tile([128, 2], F32, tag="mv")
                nc.vector.bn_aggr(mv[:], stats[:])
                mean = mv[:, 0:1]
                var = mv[:, 1:2]
                if w == NW - 1:
                    # tail path: ACT Sqrt + fast reciprocal (short latency; no
                    # exp follows, so the table switch costs nothing extra)
                    eps_t = spool.tile([128, 1], F32, tag="eps_t", bufs=1)
                    nc.vector.memset(eps_t[:], EPS)
                    sdt = spool.tile([128, 1], F32, tag="sdt")
                    nc.scalar.activation(sdt[:], var, SQRT, bias=eps_t[:])
                    y = spool.tile([128, 1], F32, tag="rstd")
                    scr_ = spool.tile([128, 1], F32, tag="scr_")
                    nc.vector.reciprocal_approx_accurate(y[:], sdt[:], scr_[:])
                    y = y[:]
                else:
                    # rstd = 1/sqrt(var): magic seed + 2 Newton steps (all
                    # DVE, keeps the ACT engine on the Exp table)
                    ish = spool.tile([128, 1], I32, tag="ish")
                    nc.vector.tensor_scalar(ish[:], var.bitcast(I32), 1, None, SHR)
                    noti = spool.tile([128, 1], I32, tag="noti")
                    nc.vector.tensor_scalar(noti[:], ish[:], -1, None, XOR)
                    seed = spool.tile([128, 1], I32, tag="seed")
                    nc.vector.tensor_scalar(seed[:], noti[:], RSQRT_MAGIC + 1, None, ADD)
                    y = seed[:].bitcast(F32)
                    for it in range(2):
                        a_ = spool.tile([128, 1], F32, tag=f"nra{it}", name=f"nra{it}")
                        nc.vector.tensor_tensor(out=a_[:], in0=y, in1=y, op=MULT)
                        b_ = spool.tile([128, 1], F32, tag=f"nrb{it}", name=f"nrb{it}")
                        nc.vector.tensor_tensor(out=b_[:], in0=a_[:], in1=var, op=MULT)
                        c_ = spool.tile([128, 1], F32, tag=f"nrc{it}", name=f"nrc{it}")
                        nc.vector.tensor_scalar(c_[:], b_[:], -0.5, 1.5, MULT, ADD)
                        y2 = spool.tile([128, 1], F32, tag=f"nry{it}", name=f"nry{it}")
                        nc.vector.tensor_tensor(out=y2[:], in0=y, in1=c_[:], op=MULT)
                        y = y2[:]
                mr = spool.tile([128, 1], F32, tag="mr")
                nc.vector.tensor_tensor(out=mr[:], in0=mean, in1=y, op=MULT)
                negmr = spool.tile([128, 1], F32, tag="negmr")
                nc.vector.tensor_scalar(negmr[:], mr[:], -1.0, None, MULT)
                onrm = lpool.tile([128, D], F32, tag="onrm")
                nc.scalar.activation(onrm[:], osb[:], IDENT, bias=negmr[:], scale=y)
                ow = lpool.tile([128, D], F32, tag="ow")
                nc.vector.tensor_tensor(out=ow[:], in0=onrm[:], in1=lnw[:], op=MULT)
                ofin = lpool.tile([128, D], F32, tag="ofin")
                nc.vector.tensor_tensor(out=ofin[:], in0=ow[:], in1=lnb[:], op=ADD)
                nc.scalar.dma_start(outd[128 * w : 128 * (w + 1), :], ofin[:])

            # ---- schedule: interleave so exp/collectives overlap PE work and
            # each window's out-proj hides behind the next window's attention
            phase_a(0, 4)
            rsb0 = phase_b(0)
            rsb1 = phase_b(1)
            phase_a(4, 8)
            phase_c(0, rsb0)
            rsb2 = phase_b(2)
            phase_c(1, rsb1)
            rsb3 = phase_b(3)
            phase_c(2, rsb2)
            phase_c(3, rsb3)

    nc.finalize()
    return nc


# feature permutation: partition p of the projected q/k carries original
# head-feature fperm[p]; head h's d-half i lives at partitions [32h,32h+32)
# free-slot i, which is partition block 64i + 32h of the projection output
def _fperm():
    p = np.empty(128, np.int64)
    for h in range(HPC):
        for dd in range(32):
            p[32 * h + dd] = 64 * h + dd            # d_lo at blocks 0/1
            p[64 + 32 * h + dd] = 64 * h + 32 + dd  # d_hi at blocks 2/3
    return p


def _rot_matrix(fperm):
    # rotate_half as a left-multiply in permuted coordinates
    R = np.zeros((128, 128), np.float32)
    for h in range(HPC):
        o = HD * h
        for j in range(32):
            R[o + j, o + 32 + j] = -1.0
            R[o + 32 + j, o + j] = 1.0
    Rp = R[fperm][:, fperm]
    return np.ascontiguousarray(Rp.T.astype(np.float16))


def _to_fp8(a):
    return np.ascontiguousarray(a.astype(ml_dtypes.float8_e4m3))


def kernel(hidden_states, cos, sin, Wq, bq, Wk, bk, Wv, bv, Wo, bo, ln_w, ln_b):
    global LAST_RESULTS
    hs = np.ascontiguousarray(np.asarray(hidden_states, np.float32).reshape(NSEQ, D))
    cos = np.asarray(cos, np.float32)
    sin = np.asarray(sin, np.float32)
    Wq = np.asarray(Wq, np.float32)
    bq = np.asarray(bq, np.float32)
    Wk = np.asarray(Wk, np.float32)
    bk = np.asarray(bk, np.float32)
    Wv = np.asarray(Wv, np.float32)
    bv = np.asarray(bv, np.float32)
    Wo = np.asarray(Wo, np.float32)
    bo = np.asarray(bo, np.float32)
    ln_w = np.asarray(ln_w, np.float32)
    ln_b = np.asarray(ln_b, np.float32)

    fperm = _fperm()
    xT8 = _to_fp8(hs.T)
    cosT = cos.T
    sinT = sin.T
    # row p carries cos/sin of d = fperm[p] % 64 (same table for both heads)
    cs2 = np.ascontiguousarray(cosT[fperm % 64].astype(np.float16))
    sn2 = np.ascontiguousarray(sinT[fperm % 64].astype(np.float16))
    lnw_t = np.ascontiguousarray(np.tile(ln_w[None, :], (128, 1)))
    lnb_t = np.ascontiguousarray(np.tile(ln_b[None, :], (128, 1)))
    woT16 = np.ascontiguousarray((Wo.T / WSCALE).astype(np.float16))
    rt = _rot_matrix(fperm)

    def w8_layout(WT_cols):
        # WT_cols: [D, 128] = W.T slice for this core's features (scaled)
        # -> [128 part, (4 cpair, 2 half, 128 feat)] fp8
        a = WT_cols.reshape(4, 2, 128, 128)          # [cp, i, p, f]
        a = a.transpose(2, 0, 1, 3)                   # [p, cp, i, f]
        return _to_fp8(a.reshape(128, D))

    resid_full = hs + bo[None, :]

    in_maps = []
    for c in range(NC):
        fs = np.arange(F * c, F * (c + 1))
        fs_p = fs[0] + fperm                          # permuted q/k features
        rows = np.concatenate(
            [resid_full[1024 * w + 128 * c : 1024 * w + 128 * (c + 1)] for w in range(NW)],
            axis=0,
        )
        in_maps.append(
            {
                "xT": xT8,
                "wq": w8_layout(WSCALE * Wq.T[:, fs_p]),
                "wk": w8_layout(WSCALE * Wk.T[:, fs_p]),
                "wv": w8_layout(WSCALE * Wv.T[:, fs]),
                "bq": np.ascontiguousarray(WSCALE * bq[fs_p, None]),
                "bk": np.ascontiguousarray(WSCALE * bk[fs_p, None]),
                "bv": np.ascontiguousarray(WSCALE * bv[fs, None]),
                "rt": rt,
                "woT": woT16,
                "cs2": cs2,
                "sn2": sn2,
                "resid": np.ascontiguousarray(rows),
                "lnw": lnw_t,
                "lnb": lnb_t,
            }
        )

    nc = _build()
    LAST_RESULTS = run_bass_kernel_spmd(nc, in_maps, core_ids=list(range(NC)))
    out = np.empty((NSEQ, D), np.float32)
    for c in range(NC):
        res = LAST_RESULTS.results[c]["out"]
        for w in range(NW):
            out[1024 * w + 128 * c : 1024 * w + 128 * (c + 1)] = res[
                128 * w : 128 * (w + 1)
            ]
    return out.reshape(B, S, D)
